# revision 1
# baseline (speedup 1.0000x reference)
"""Causal self-attention (B=4, T=2048, C=1024, H=16) on 8 trn2 NeuronCores.

Sharding: head-pair parallel. Core c owns heads {2c, 2c+1} for all 4 batches.
 - host: x is pre-transposed to xT [C, B*T]; W_qkv is pre-sliced per core into
   wq/wk/wv [C, 128] (2 heads x 64, softmax scale folded into wq), W_proj and
   biases broadcast.
 - device per core: qkv projections as fp32r matmuls producing qT/kT [d2, T]
   (d on partitions) and vT [d2, T]; vT is PE-transposed per 128-tile into
   v [T, 64]-per-head tiles with an appended ones column.
 - attention in S^T orientation: S^T[tk, tq] = kT.T@qT tiles [128, 512] with the
   causal mask preloaded into PSUM via an identity matmul; softmax without
   max-subtraction (|S| <= ~20, safe in fp32): P^T = exp(S^T) on ScalarE
   (PSUM->SBUF, rounded to f32r). O-matmul lhsT = [v_h | ones] (M=65) yields
   both O^T[d, tq] and the denominator row l in one pass. Normalize via
   reciprocal + K=1 broadcast matmul + DVE multiply.
 - per-batch AllToAll (1 MB/rank) reshards O^T from head-shards to
   token-shards; column-parallel out-projection with fused bias produces
   out^T [C, 1024 tokens] per core; host reassembles.
"""
import numpy as np
import concourse.bacc as bacc
import concourse.mybir as mybir
import concourse.tile as tile
from concourse.bass_utils import run_bass_kernel_spmd
from concourse.masks import make_identity

F32 = mybir.dt.float32
F32R = mybir.dt.float32r
Exp = mybir.ActivationFunctionType.Exp

NCORES = 8
B, T, C, H = 4, 2048, 1024, 16
HD = C // H          # 64
HL = H // NCORES     # 2 heads per core
D2 = HL * HD         # 128 rows of local head-pair dims
TB = T               # tokens per batch
NKC = C // 128       # 8 contraction chunks
NCH = TB // 512      # 4 tq chunks per batch
NTK = TB // 128      # 16 tk tiles per batch
PIECE = TB // NCORES  # 256 tokens per (batch, core) piece after AllToAll

_CACHE = {}


def _build(sim=False):
    nc = bacc.Bacc("TRN2", target_bir_lowering=False, debug=False,
                   num_devices=1 if sim else NCORES)
    xt = nc.dram_tensor("xt", [C, B * T], F32R, kind="ExternalInput").ap()
    wq = nc.dram_tensor("wq", [C, D2], F32R, kind="ExternalInput").ap()
    wk = nc.dram_tensor("wk", [C, D2], F32R, kind="ExternalInput").ap()
    wv = nc.dram_tensor("wv", [C, D2], F32R, kind="ExternalInput").ap()
    wp = nc.dram_tensor("wp", [C, C], F32R, kind="ExternalInput").ap()
    bqkv = nc.dram_tensor("bqkv", [D2, 3], F32, kind="ExternalInput").ap()
    bp = nc.dram_tensor("bp", [128, NKC], F32, kind="ExternalInput").ap()
    outp = nc.dram_tensor("outp", [C, B * PIECE], F32, kind="ExternalOutput").ap()

    inb = [nc.dram_tensor(f"inb{b}", [NCORES, D2, PIECE], F32R) for b in range(B)]
    outb = [nc.dram_tensor(f"outb{b}", [NCORES, D2, PIECE], F32R) for b in range(B)]

    with tile.TileContext(nc) as tc:
        with (
            tc.tile_pool(name="const", bufs=1) as cpool,
            tc.tile_pool(name="w", bufs=1) as wpool,
            tc.tile_pool(name="xt", bufs=16) as xpool,
            tc.tile_pool(name="qk", bufs=2) as qkpool,
            tc.tile_pool(name="vstg", bufs=1) as vstgpool,
            tc.tile_pool(name="vh", bufs=2) as vhpool,
            tc.tile_pool(name="pt", bufs=5) as ptpool,
            tc.tile_pool(name="small", bufs=3) as smallpool,
            tc.tile_pool(name="ofin", bufs=4) as ofinpool,
            tc.tile_pool(name="proj", bufs=3) as projpool,
            tc.tile_pool(name="otp", bufs=9) as otpool,
            tc.tile_pool(name="mm", bufs=2, space="PSUM") as mmps,
            tc.tile_pool(name="s", bufs=2, space="PSUM") as sps,
            tc.tile_pool(name="o", bufs=1, space="PSUM") as ops,
            
        ):
            # ---- constants ----
            ident32 = cpool.tile([128, 128], F32)
            make_identity(nc, ident32[:])
            idr = cpool.tile([128, 128], F32R)
            mask32 = cpool.tile([128, 512], F32)
            masks = cpool.tile([128, 4 * 512], F32R)
            ones32 = cpool.tile([128, 16], F32)
            ones64 = cpool.tile([1, 64], F32)
            onesr = cpool.tile([1, 64], F32R)
            nc.gpsimd.memset(ones32[:], 1.0)
            nc.gpsimd.memset(ones64[:], 1.0)
            with nc.allow_low_precision(reason="f32r operand staging"):
                nc.vector.tensor_copy(idr[:], ident32[:])
                nc.vector.tensor_copy(onesr[:], ones64[:])
                for m in range(4):
                    nc.gpsimd.memset(mask32[:], 0.0)
                    # keep where tq_local >= tk_local + 128*m
                    nc.gpsimd.affine_select(
                        out=mask32[:], in_=mask32[:],
                        compare_op=mybir.AluOpType.is_ge, fill=-1e30,
                        base=-128 * m, channel_multiplier=-1,
                        pattern=[[1, 512]],
                    )
                    nc.vector.tensor_copy(masks[:, 512 * m:512 * (m + 1)],
                                          mask32[:])

            # ---- weights ----
            wq_sb = wpool.tile([128, NKC, D2], F32R)
            wk_sb = wpool.tile([128, NKC, D2], F32R)
            wv_sb = wpool.tile([128, NKC, D2], F32R)
            for t, d in ((wq_sb, wq), (wk_sb, wk), (wv_sb, wv)):
                nc.sync.dma_start(
                    t[:], d.rearrange("(kc p) m -> p kc m", p=128))
            wp_sb = wpool.tile([128, NKC, C], F32R)
            nc.sync.dma_start(
                wp_sb[:], wp.rearrange("(kc p) m -> p kc m", p=128))
            bqkv_sb = cpool.tile([D2, 3], F32)
            nc.sync.dma_start(bqkv_sb[:], bqkv)
            bp_sb = cpool.tile([128, NKC], F32)
            nc.sync.dma_start(bp_sb[:], bp)

            for b in range(B):
                g0 = b * TB
                # ---- qkv projections ----
                qT = qkpool.tile([D2, TB], F32R, tag="qT")
                kT = qkpool.tile([D2, TB], F32R, tag="kT")
                vT = vstgpool.tile([D2, TB], F32)
                for n in range(NCH):
                    xts = []
                    for kc in range(NKC):
                        xtile = xpool.tile([128, 512], F32R)
                        nc.sync.dma_start(
                            xtile[:],
                            xt[128 * kc:128 * (kc + 1),
                               g0 + 512 * n:g0 + 512 * (n + 1)])
                        xts.append(xtile)
                    for w_sb, col in ((wq_sb, 0), (wk_sb, 1), (wv_sb, 2)):
                        ps = mmps.tile([128, 512], F32, tag="ps")
                        for kc in range(NKC):
                            nc.tensor.matmul(
                                ps[:], w_sb[:, kc, :],
                                xts[kc][:], start=(kc == 0),
                                stop=(kc == NKC - 1))
                        dst = (qT, kT, vT)[col]
                        with nc.allow_low_precision(reason="f32r qkv"):
                            nc.vector.tensor_scalar_add(
                                dst[:, 512 * n:512 * (n + 1)], ps[:],
                                bqkv_sb[:, col:col + 1])

                # ---- v transposes: vT [d2, T] -> per-head v [T, 65] tiles ----
                vh = [vhpool.tile([128, NTK * 65], F32R, tag=f"vh{h}",
                                  name=f"vh{h}") for h in range(HL)]
                for h in range(HL):
                    with nc.allow_low_precision(reason="f32r v ones"):
                        nc.vector.tensor_copy(vh[h][:, 64::65], ones32[:])
                    for tk in range(NTK):
                        vt_ps = mmps.tile([128, 64], F32, tag="ps", name="vt_ps")
                        nc.tensor.transpose(
                            vt_ps[:],
                            vT[64 * h:64 * (h + 1), 128 * tk:128 * (tk + 1)],
                            ident32[64 * h:64 * (h + 1), 64 * h:64 * (h + 1)])
                        with nc.allow_low_precision(reason="f32r v"):
                            nc.vector.tensor_copy(
                                vh[h][:, 65 * tk:65 * tk + 64], vt_ps[:])

                # ---- attention per tq-chunk ----
                for j in range(NCH):
                    o_ps = [ops.tile([65, 512], F32, tag=f"o{h}", name=f"o{h}")
                            for h in range(HL)]
                    ktop = 4 * j + 4
                    for tk in range(ktop):
                        m = tk - 4 * j
                        # cols [0, z) of this tile are fully causal-masked
                        z = 128 * m if m > 0 else 0
                        w = 512 - z
                        s_ps = sps.tile([128, 1024], F32, tag="s_ps")
                        if m >= 0:
                            for h in range(HL):
                                nc.tensor.matmul(
                                    s_ps[:, 512 * h + z:512 * (h + 1)],
                                    idr[:],
                                    masks[:, 512 * m + z:512 * (m + 1)],
                                    start=True, stop=False)
                        # K=64 pair at row groups (0,0)/(64,0) -> concurrent
                        for h in range(HL):
                            nc.tensor.matmul(
                                s_ps[:, 512 * h + z:512 * (h + 1)],
                                kT[64 * h:64 * (h + 1),
                                   128 * tk:128 * (tk + 1)],
                                qT[64 * h:64 * (h + 1),
                                   512 * j + z:512 * (j + 1)],
                                start=(m < 0), stop=True)
                        pt = ptpool.tile([128, 1024], F32R, tag="pt")
                        if z:
                            exp_src = s_ps[:].rearrange(
                                "p (g c) -> p g c", g=2)[:, :, z:]
                            exp_dst = pt[:].rearrange(
                                "p (g c) -> p g c", g=2)[:, :, z:]
                            nc.scalar.activation(exp_dst, exp_src, Exp)
                        else:
                            nc.scalar.activation(pt[:], s_ps[:], Exp)
                        for h in range(HL):
                            nc.tensor.matmul(
                                o_ps[h][0:65, z:512],
                                vh[h][:, 65 * tk:65 * (tk + 1)],
                                pt[:, 512 * h + z:512 * (h + 1)],
                                start=(tk == 0), stop=(tk == ktop - 1))
                    for h in range(HL):
                        o_sb = smallpool.tile([65, 512], F32, tag="osb2")
                        nc.vector.tensor_copy(o_sb[:], o_ps[h][:])
                        r_sb = smallpool.tile([1, 512], F32R, tag="r")
                        with nc.allow_low_precision(reason="softmax denom"):
                            nc.vector.reciprocal(r_sb[:], o_sb[64:65, :])
                        rb_ps = mmps.tile([64, 512], F32, tag="ps", name="rb_ps")
                        nc.tensor.matmul(rb_ps[:], onesr[:], r_sb[:],
                                         start=True, stop=True)
                        rb_sb = smallpool.tile([64, 512], F32, tag="rb")
                        nc.vector.tensor_copy(rb_sb[:], rb_ps[:])
                        ofin = ofinpool.tile([64, 512], F32R)
                        with nc.allow_low_precision(reason="f32r O"):
                            nc.gpsimd.tensor_mul(ofin[:], o_sb[0:64, :],
                                                 rb_sb[:])
                        for half in range(2):
                            s8 = 2 * j + half
                            nc.sync.dma_start(
                                inb[b].ap()[s8, 64 * h:64 * (h + 1), :],
                                ofin[:, 256 * half:256 * (half + 1)])

                # ---- AllToAll: head-shards -> token-shards ----
                if sim:
                    # stand-in with comparable cost for the cost-model sim
                    nc.sync.dma_start(outb[b].ap(), inb[b].ap())
                else:
                    nc.gpsimd.collective_compute(
                        "AllToAll", mybir.AluOpType.bypass,
                        replica_groups=[list(range(NCORES))],
                        ins=[inb[b].ap().opt()], outs=[outb[b].ap().opt()],
                    )

                # ---- out projection (column-parallel, out^T) ----
                ots = []
                for s8 in range(NCORES):
                    ot = otpool.tile([128, PIECE], F32R, tag="ot")
                    nc.sync.dma_start(ot[:], outb[b].ap()[s8])
                    ots.append(ot)
                for mcol in range(NKC):
                    pp = mmps.tile([128, PIECE], F32, tag="ps")
                    for s8 in range(NCORES):
                        nc.tensor.matmul(
                            pp[:],
                            wp_sb[:, s8, 128 * mcol:128 * (mcol + 1)],
                            ots[s8][:], start=(s8 == 0),
                            stop=(s8 == NCORES - 1))
                    osb = projpool.tile([128, PIECE], F32, tag="osb")
                    nc.vector.tensor_scalar_add(osb[:], pp[:],
                                                bp_sb[:, mcol:mcol + 1])
                    nc.sync.dma_start(
                        outp[128 * mcol:128 * (mcol + 1),
                             PIECE * b:PIECE * (b + 1)], osb[:])
    nc.compile()
    return nc


def _get_nc():
    if "nc" not in _CACHE:
        _CACHE["nc"] = _build()
    return _CACHE["nc"]


def kernel(x, W_qkv, b_qkv, W_proj, b_proj):
    x = np.asarray(x, dtype=np.float32)
    W_qkv = np.asarray(W_qkv, dtype=np.float32)
    b_qkv = np.asarray(b_qkv, dtype=np.float32)
    W_proj = np.asarray(W_proj, dtype=np.float32)
    b_proj = np.asarray(b_proj, dtype=np.float32)

    scale = 1.0 / np.sqrt(HD)
    xt = np.ascontiguousarray(x.reshape(B * T, C).T)          # [C, B*T]
    wp = np.ascontiguousarray(W_proj)                          # [C, C]
    bp = np.ascontiguousarray(b_proj.reshape(NKC, 128).T)      # [128, 8]

    qw = W_qkv[:, 0:C]
    kw = W_qkv[:, C:2 * C]
    vw = W_qkv[:, 2 * C:3 * C]
    qb, kb, vb = b_qkv[0:C], b_qkv[C:2 * C], b_qkv[2 * C:3 * C]

    in_maps = []
    for c in range(NCORES):
        cols = slice(2 * c * HD, (2 * c + 2) * HD)  # this core's 128 dims
        bq = np.stack([qb[cols] * scale, kb[cols], vb[cols]], axis=1)  # [128,3]
        in_maps.append({
            "xt": xt,
            "wq": np.ascontiguousarray(qw[:, cols] * scale),
            "wk": np.ascontiguousarray(kw[:, cols]),
            "wv": np.ascontiguousarray(vw[:, cols]),
            "wp": wp,
            "bqkv": np.ascontiguousarray(bq),
            "bp": bp,
        })

    nc = _get_nc()
    _CACHE["last_in_maps"] = in_maps
    res = run_bass_kernel_spmd(nc, in_maps, core_ids=list(range(NCORES)))

    # outp[c]: [C, B*PIECE] (cols: b-major, then 256 tokens of piece c)
    allo = np.stack([res.results[c]["outp"] for c in range(NCORES)])
    allo = allo.reshape(NCORES, C, B, PIECE)       # [c, ch, b, u]
    out = allo.transpose(2, 0, 3, 1).reshape(B, T, C)
    return np.ascontiguousarray(out)



# revision 34
# speedup vs baseline: 1.5453x; 1.5453x over previous
"""Causal self-attention (B=4, T=2048, C=1024, H=16) on 8 trn2 NeuronCores.

Sharding: head-pair parallel. Core c owns heads {2c, 2c+1} for all 4 batches.
 - host: x is pre-transposed to xT [C, B*T] (bf16); W_qkv is pre-sliced per
   core into wq/wk/wv [C, 128] bf16 (softmax scale folded into wq), W_proj
   (f32r) and biases broadcast.
 - device per core: qkv projections as bf16 matmuls producing qT/kT [d2, T]
   (d on partitions, bf16) and vT [d2, T]; vT is PE-transposed per 128-tile
   (both heads in one transpose) into v [T, 64]-per-head bf16 tiles with an
   appended ones column.
 - attention in S^T orientation: S^T[tk, tq] = kT.T@qT tiles [128, 1024]
   (both heads side by side) in PSUM; softmax without max-subtraction (|S|
   small, safe in fp32): P^T = exp(S^T) on ScalarE (PSUM->SBUF, bf16).
   Causal mask applied only on the 128-wide diagonal blocks by multiplying
   P^T with a precomputed 0/1 triangular bf16 tile on the vector engine (no
   PE mask matmuls). O-matmul lhsT = [v_h | ones] (M=65) yields both
   O^T[d, tq] and the denominator row l in one pass. Normalize via DVE
   reciprocal + gpsimd partition_broadcast + DVE multiply (no PE broadcast
   matmul).
 - the attention loop is software-pipelined (S of tile t+1 ahead of O of
   tile t, double-buffered PSUM) and the next batch's x loads / qkv chains
   are interleaved per tq-chunk so the tensor engine stays busy during the
   scalar-engine-bound stretches.
 - per-batch AllToAll (1 MB/rank) reshards O^T from head-shards to
   token-shards; column-parallel out-projection with fused bias produces
   out^T [C, 1024 tokens] per core; host reassembles. Projections are
   deferred one batch so the tensor engine never waits on a collective.
"""
import numpy as np
import ml_dtypes
import concourse.bacc as bacc
import concourse.mybir as mybir
import concourse.tile as tile
from concourse.bass_utils import run_bass_kernel_spmd
from concourse.masks import make_identity

F32 = mybir.dt.float32
F32R = mybir.dt.float32r
BF16 = mybir.dt.bfloat16
Exp = mybir.ActivationFunctionType.Exp

NCORES = 8
B, T, C, H = 4, 2048, 1024, 16
HD = C // H          # 64
HL = H // NCORES     # 2 heads per core
D2 = HL * HD         # 128 rows of local head-pair dims
TB = T               # tokens per batch
NKC = C // 128       # 8 contraction chunks
NCH = TB // 512      # 4 tq chunks per batch
NTK = TB // 128      # 16 tk tiles per batch
PIECE = TB // NCORES  # 256 tokens per (batch, core) piece after AllToAll

_CACHE = {}


def _build(sim=False):
    nc = bacc.Bacc("TRN2", target_bir_lowering=False, debug=False,
                   num_devices=1 if sim else NCORES)
    xt = nc.dram_tensor("xt", [C, B * T], BF16, kind="ExternalInput").ap()
    wq = nc.dram_tensor("wq", [C, D2], BF16, kind="ExternalInput").ap()
    wk = nc.dram_tensor("wk", [C, D2], BF16, kind="ExternalInput").ap()
    wv = nc.dram_tensor("wv", [C, D2], BF16, kind="ExternalInput").ap()
    wp = nc.dram_tensor("wp", [C, C], F32R, kind="ExternalInput").ap()
    bqkv = nc.dram_tensor("bqkv", [D2, 3], F32, kind="ExternalInput").ap()
    bp = nc.dram_tensor("bp", [128, NKC], F32, kind="ExternalInput").ap()
    outp = nc.dram_tensor("outp", [C, B * PIECE], F32, kind="ExternalOutput").ap()

    inb = [nc.dram_tensor(f"inb{b}", [NCORES, D2, PIECE], F32R) for b in range(B)]
    outb = [nc.dram_tensor(f"outb{b}", [NCORES, D2, PIECE], F32R) for b in range(B)]

    with tile.TileContext(nc) as tc:
        with (
            tc.tile_pool(name="const", bufs=1) as cpool,
            tc.tile_pool(name="w", bufs=1) as wpool,
            tc.tile_pool(name="xt", bufs=5) as xpool,
            tc.tile_pool(name="qk", bufs=2) as qkpool,
            tc.tile_pool(name="vstg", bufs=2) as vstgpool,
            tc.tile_pool(name="vh", bufs=2) as vhpool,
            tc.tile_pool(name="pt", bufs=5) as ptpool,
            tc.tile_pool(name="small", bufs=2) as smallpool,
            tc.tile_pool(name="ofin", bufs=4) as ofinpool,
            tc.tile_pool(name="ot", bufs=1) as otpool,
            tc.tile_pool(name="proj", bufs=1) as projpool,
            tc.tile_pool(name="mm", bufs=2, space="PSUM") as mmps,
            tc.tile_pool(name="s", bufs=2, space="PSUM") as sps,
            tc.tile_pool(name="o", bufs=1, space="PSUM") as ops,
        ):
            # ---- constants ----
            ident32 = cpool.tile([128, 128], F32)
            make_identity(nc, ident32[:])
            identb = cpool.tile([128, 128], BF16)
            tri32 = cpool.tile([128, 128], F32)
            trib = cpool.tile([128, 128], BF16)
            onesb = cpool.tile([128, 16], BF16)
            nc.gpsimd.memset(onesb[:], 1.0)
            nc.gpsimd.memset(tri32[:], 1.0)
            # keep where tq_local >= tk_local (lower-left in S^T layout)
            nc.gpsimd.affine_select(
                out=tri32[:], in_=tri32[:],
                compare_op=mybir.AluOpType.is_ge, fill=0.0,
                base=0, channel_multiplier=-1,
                pattern=[[1, 128]],
            )
            with nc.allow_low_precision(reason="bf16 constant staging"):
                nc.vector.tensor_copy(identb[:], ident32[:])
                nc.vector.tensor_copy(trib[:], tri32[:])

            # ---- weights ----
            # wv first (the first qkv chain is v); biases off the SP queue so
            # they don't delay the startup-critical x load issue
            wq_sb = wpool.tile([128, NKC, D2], BF16)
            wk_sb = wpool.tile([128, NKC, D2], BF16)
            wv_sb = wpool.tile([128, NKC, D2], BF16)
            nc.sync.dma_start(
                wv_sb[:], wv.rearrange("(kc p) m -> p kc m", p=128))
            bqkv_sb = cpool.tile([D2, 3], F32)
            nc.gpsimd.dma_start(bqkv_sb[:], bqkv)
            bp_sb = cpool.tile([128, NKC], F32)
            nc.gpsimd.dma_start(bp_sb[:], bp)

            def emit_x_load(b, n, split=False):
                xts = xpool.tile([128, NKC, 512], BF16, tag="x")
                src = xt[:, b * TB + 512 * n:b * TB + 512 * (n + 1)].rearrange(
                    "(kc p) m -> p kc m", p=128)
                if split:
                    # spread the latency-critical first load across queues,
                    # issued from the idle scalar engine
                    for kc in range(0, NKC, 2):
                        nc.scalar.dma_start(xts[:, kc:kc + 2, :],
                                            src[:, kc:kc + 2, :])
                else:
                    nc.sync.dma_start(xts[:], src)
                return xts

            def alloc_qkv(b):
                qT = qkpool.tile([D2, TB], BF16, tag="qT")
                kT = qkpool.tile([D2, TB], BF16, tag="kT")
                vT = vstgpool.tile([D2, TB], BF16, tag="vT")
                vh = [vhpool.tile([128, NTK * 65], BF16, tag=f"vh{h}",
                                  name=f"vh{h}") for h in range(HL)]
                for h in range(HL):
                    nc.gpsimd.tensor_copy(vh[h][:, 64::65], onesb[:])
                return qT, kT, vT, vh

            def emit_qkv_chain(st, n, col, part=None, cell=None):
                xts, qT, kT, vT, vh = st[:5]
                w_sb = (wq_sb, wk_sb, wv_sb)[col]
                if part in (None, 0):
                    cell = cell if cell is not None else {}
                    cell['ps'] = mmps.tile([128, 512], F32, tag="ps",
                                           name="qkv_ps")
                ps = cell['ps']
                kcs = (range(NKC) if part is None else
                       range(part * NKC // 2, (part + 1) * NKC // 2))
                for kc in kcs:
                    nc.tensor.matmul(
                        ps[:], w_sb[:, kc, :],
                        xts[n][:, kc, :], start=(kc == 0),
                        stop=(kc == NKC - 1))
                if part in (None, 1):
                    dst = (qT, kT, vT)[col]
                    with nc.allow_low_precision(reason="bf16 qkv"):
                        nc.vector.tensor_scalar_add(
                            dst[:, 512 * n:512 * (n + 1)], ps[:],
                            bqkv_sb[:, col:col + 1])

            def emit_v_transpose(st, tk):
                # both heads in one PE transpose per tk tile
                vT, vh = st[3], st[4]
                vt_ps = mmps.tile([128, 128], BF16, tag="ps", name="vt_ps")
                nc.tensor.transpose(
                    vt_ps[:], vT[:, 128 * tk:128 * (tk + 1)], identb[:])
                for h in range(HL):
                    # (DVE, not gpsimd: GPSIMD cannot access PSUM)
                    with nc.allow_low_precision(reason="bf16 v"):
                        nc.vector.tensor_copy(
                            vh[h][:, 65 * tk:65 * tk + 64],
                            vt_ps[:, 64 * h:64 * (h + 1)])

            def qkv_units(st):
                """Filler units: half a qkv chain or one transpose each,
                chunk-major. v first so its eviction (DVE) overlaps the q/k
                chains."""
                for n in range(NCH):
                    for col in (2, 0, 1):
                        cell = {}
                        for part in (0, 1):
                            yield (lambda n=n, col=col, part=part, cell=cell:
                                   emit_qkv_chain(st, n, col, part, cell))
                    for tk in range(4 * n, 4 * n + 4):
                        yield lambda tk=tk: emit_v_transpose(st, tk)

            def emit_qkv_chunk(st, n):
                for col in (2, 0, 1):
                    emit_qkv_chain(st, n, col)
                for tk in range(4 * n, 4 * n + 4):
                    emit_v_transpose(st, tk)

            def emit_attention(b, qT, kT, vh, units):
                units = list(units)
                n_tiles = sum(4 * j + 4 for j in range(NCH))
                tile_i = 0
                emitted = 0

                def pace():
                    nonlocal emitted
                    want = (tile_i * len(units)) // n_tiles
                    while emitted < want:
                        units[emitted]()
                        emitted += 1

                for j in range(NCH):
                    o_ps = [ops.tile([65, 512], F32, tag=f"o{h}", name=f"o{h}")
                            for h in range(HL)]
                    ktop = 4 * j + 4
                    pts = [None] * ktop  # (pt, z) per tile

                    def emit_o(t):
                        pt, z = pts[t]
                        for h in range(HL):
                            nc.tensor.matmul(
                                o_ps[h][0:65, z:512],
                                vh[h][:, 65 * t:65 * (t + 1)],
                                pt[:, 512 * h + z:512 * (h + 1)],
                                start=(t == 0), stop=(t == ktop - 1))

                    for t in range(ktop):
                        m = t - 4 * j
                        z = 128 * m if m > 0 else 0
                        s_ps = sps.tile([128, 1024], F32, tag="s")
                        for h in range(HL):
                            nc.tensor.matmul(
                                s_ps[:, 512 * h + z:512 * (h + 1)],
                                kT[64 * h:64 * (h + 1),
                                   128 * t:128 * (t + 1)],
                                qT[64 * h:64 * (h + 1),
                                   512 * j + z:512 * (j + 1)],
                                start=True, stop=True)
                        pt = ptpool.tile([128, 1024], BF16, tag="pt")
                        if z:
                            exp_src = s_ps[:].rearrange(
                                "p (g c) -> p g c", g=2)[:, :, z:]
                            exp_dst = pt[:].rearrange(
                                "p (g c) -> p g c", g=2)[:, :, z:]
                            nc.scalar.activation(exp_dst, exp_src, Exp)
                        else:
                            nc.scalar.activation(pt[:], s_ps[:], Exp)
                        if m >= 0:
                            # zero the upper triangle of the diagonal block
                            for h in range(HL):
                                nc.vector.tensor_mul(
                                    pt[:, 512 * h + z:512 * h + z + 128],
                                    pt[:, 512 * h + z:512 * h + z + 128],
                                    trib[:])
                        pts[t] = (pt, z)
                        if t >= 1:
                            emit_o(t - 1)
                        tile_i += 1
                        pace()
                    emit_o(ktop - 1)

                    # ---- normalization ----
                    last = b == B - 1 and j == NCH - 1
                    for h in range(HL):
                        if last:
                            # tail-latency path: no successor needs the PSUM
                            # bank, so skip the eviction copy
                            o_sb = o_ps[h]
                        else:
                            # evict PSUM immediately so the next chunk's O
                            # accumulation isn't blocked on the norm chain
                            o_sb = smallpool.tile([65, 512], F32, tag="osb")
                            nc.vector.tensor_copy(o_sb[:], o_ps[h][:])
                        r_sb = smallpool.tile([1, 512], F32, tag="r")
                        nc.vector.reciprocal(r_sb[:], o_sb[64:65, :])
                        rb_sb = smallpool.tile([64, 512], F32, tag="rb")
                        nc.gpsimd.partition_broadcast(rb_sb[:], r_sb[:])
                        ofin = ofinpool.tile([64, 512], F32R)
                        with nc.allow_low_precision(reason="f32r O"):
                            nc.vector.tensor_mul(ofin[:], o_sb[0:64, :],
                                                 rb_sb[:])
                        for half in range(2):
                            s8 = 2 * j + half
                            nc.sync.dma_start(
                                inb[b].ap()[s8, 64 * h:64 * (h + 1), :],
                                ofin[:, 256 * half:256 * (half + 1)])
                while emitted < len(units):
                    units[emitted]()
                    emitted += 1

            def emit_collective(b):
                if sim:
                    # stand-in for the cost-model sim: the resharding data
                    # movement as per-destination slice copies (parallel DMA
                    # rings, like the real AllToAll); wire time excluded
                    for s8 in range(NCORES):
                        nc.gpsimd.dma_start(outb[b].ap()[s8], inb[b].ap()[s8])
                else:
                    nc.gpsimd.collective_compute(
                        "AllToAll", mybir.AluOpType.bypass,
                        replica_groups=[list(range(NCORES))],
                        ins=[inb[b].ap().opt()], outs=[outb[b].ap().opt()],
                    )

            def emit_proj_load(bs, split=False):
                nb = len(bs)
                tag = "ot" + "".join(map(str, bs))
                ot = otpool.tile([128, NCORES, nb * PIECE], F32R, tag=tag)
                for i in range(nb):
                    src = outb[bs[i]].ap().rearrange("s p m -> p s m")
                    if split:
                        # tail-latency path: parallel queues, issued from the
                        # idle scalar engine
                        for s8 in range(0, NCORES, 2):
                            nc.scalar.dma_start(
                                ot[:, s8:s8 + 2, PIECE * i:PIECE * (i + 1)],
                                src[:, s8:s8 + 2, :])
                    else:
                        nc.sync.dma_start(
                            ot[:, :, PIECE * i:PIECE * (i + 1)], src)
                osb = projpool.tile([128, NKC, nb * PIECE], F32,
                                    tag="osb" + "".join(map(str, bs)))
                return ot, osb

            def emit_proj_cols(bs, ot, osb, mcols, tail=False):
                # out-projection matmuls for one or two batches (512-wide
                # when two)
                w = len(bs) * PIECE
                for mcol in mcols:
                    pp = mmps.tile([128, 512], F32, tag="ps", name="pp")
                    for s8 in range(NCORES):
                        nc.tensor.matmul(
                            pp[:, 0:w],
                            wp_sb[:, s8, 128 * mcol:128 * (mcol + 1)],
                            ot[:, s8, :], start=(s8 == 0),
                            stop=(s8 == NCORES - 1))
                    # (DVE, not gpsimd: GPSIMD cannot access PSUM)
                    nc.vector.tensor_scalar_add(osb[:, mcol, :], pp[:, 0:w],
                                                bp_sb[:, mcol:mcol + 1])

            def emit_proj_store(bs, osb, nsplit=1, tail=False):
                # split the store across queues to shorten the tail
                eng = nc.scalar if tail else nc.sync
                for i in range(len(bs)):
                    dst = outp[:, PIECE * bs[i]:PIECE * (bs[i] + 1)].rearrange(
                        "(mc p) m -> p mc m", p=128)
                    step = NKC // nsplit
                    for k in range(nsplit):
                        eng.dma_start(
                            dst[:, k * step:(k + 1) * step, :],
                            osb[:, k * step:(k + 1) * step,
                                PIECE * i:PIECE * (i + 1)])

            def emit_proj(bs):
                ot, osb = emit_proj_load(bs)
                emit_proj_cols(bs, ot, osb, range(NKC))
                emit_proj_store(bs, osb)

            # ---- main schedule ----
            states = [None] * B

            def make_state(b):
                xts = [emit_x_load(b, n, split=(b == 0 and n == 0))
                       for n in range(NCH)]
                return [xts, *alloc_qkv(b)]

            xts0 = [emit_x_load(0, 0, split=True)]
            for t, d in ((wq_sb, wq), (wk_sb, wk)):
                nc.sync.dma_start(
                    t[:], d.rearrange("(kc p) m -> p kc m", p=128))
            xts0 += [emit_x_load(0, n) for n in range(1, NCH)]
            states[0] = [xts0, *alloc_qkv(0)]
            for n in range(NCH):
                emit_qkv_chunk(states[0], n)
            states[1] = make_state(1)
            # W_proj load deferred past the latency-critical start
            wp_sb = wpool.tile([128, NKC, C], F32R)
            nc.sync.dma_start(
                wp_sb[:], wp.rearrange("(kc p) m -> p kc m", p=128))

            proj01 = [None, None]
            proj2 = [None, None]
            for b in range(B):
                if b + 1 < B:
                    units = qkv_units(states[b + 1])
                else:
                    # spread the deferred projections of batches 0-2 into
                    # batch 3's attention as tensor-engine gap filler
                    def proj_units():
                        for mcol in range(NKC):
                            yield (lambda mcol=mcol: emit_proj_cols(
                                [0, 1], proj01[0], proj01[1], [mcol]))
                            yield (lambda mcol=mcol: emit_proj_cols(
                                [2], proj2[0], proj2[1], [mcol]))
                    units = proj_units()
                st = states[b]
                emit_attention(b, st[1], st[2], st[4], units)
                emit_collective(b)
                if b + 2 < B:
                    states[b + 2] = make_state(b + 2)
                if b == 1:
                    proj01[0], proj01[1] = emit_proj_load([0, 1])
                elif b == 2:
                    proj2[0], proj2[1] = emit_proj_load([2])
            emit_proj_store([0, 1], proj01[1])
            emit_proj_store([2], proj2[1])
            ot3, osb3 = emit_proj_load([3], split=True)
            emit_proj_cols([3], ot3, osb3, range(NKC), tail=True)
            emit_proj_store([3], osb3, nsplit=4, tail=True)
    nc.compile()
    return nc


def _get_nc():
    if "nc" not in _CACHE:
        _CACHE["nc"] = _build()
    return _CACHE["nc"]


def kernel(x, W_qkv, b_qkv, W_proj, b_proj):
    x = np.asarray(x, dtype=np.float32)
    W_qkv = np.asarray(W_qkv, dtype=np.float32)
    b_qkv = np.asarray(b_qkv, dtype=np.float32)
    W_proj = np.asarray(W_proj, dtype=np.float32)
    b_proj = np.asarray(b_proj, dtype=np.float32)

    scale = 1.0 / np.sqrt(HD)
    BFD = ml_dtypes.bfloat16
    xtb = np.ascontiguousarray(x.reshape(B * T, C).T).astype(BFD)  # [C, B*T]
    wp = np.ascontiguousarray(W_proj)                          # [C, C]
    bpz = np.ascontiguousarray(b_proj.reshape(NKC, 128).T)     # [128, 8]

    qw = W_qkv[:, 0:C]
    kw = W_qkv[:, C:2 * C]
    vw = W_qkv[:, 2 * C:3 * C]
    qb, kb, vb = b_qkv[0:C], b_qkv[C:2 * C], b_qkv[2 * C:3 * C]

    in_maps = []
    for c in range(NCORES):
        cols = slice(2 * c * HD, (2 * c + 2) * HD)  # this core's 128 dims
        bq = np.stack([qb[cols] * scale, kb[cols], vb[cols]], axis=1)  # [128,3]
        in_maps.append({
            "xt": xtb,
            "wq": np.ascontiguousarray(qw[:, cols] * scale).astype(BFD),
            "wk": np.ascontiguousarray(kw[:, cols]).astype(BFD),
            "wv": np.ascontiguousarray(vw[:, cols]).astype(BFD),
            "wp": wp,
            "bqkv": np.ascontiguousarray(bq),
            "bp": bpz,
        })

    nc = _get_nc()
    _CACHE["last_in_maps"] = in_maps
    res = run_bass_kernel_spmd(nc, in_maps, core_ids=list(range(NCORES)))

    # outp[c]: [C, B*PIECE] (cols: b-major, then 256 tokens of piece c)
    allo = np.stack([res.results[c]["outp"] for c in range(NCORES)])
    allo = allo.reshape(NCORES, C, B, PIECE)       # [c, ch, b, u]
    out = allo.transpose(2, 0, 3, 1).reshape(B, T, C)
    return np.ascontiguousarray(out)


# revision 43
# speedup vs baseline: 1.5598x; 1.0093x over previous
"""Causal self-attention (B=4, T=2048, C=1024, H=16) on 8 trn2 NeuronCores.

Sharding: head-pair parallel. Core c owns heads {2c, 2c+1} for all 4 batches.
 - host: x is pre-transposed to xT [C, B*T] (bf16); W_qkv is pre-sliced per
   core into wq/wk/wv [C, 128] bf16 (softmax scale folded into wq), W_proj
   (f32r) and biases broadcast.
 - device per core: qkv projections as bf16 matmuls producing qT/kT [d2, T]
   (d on partitions, bf16) and vT [d2, T]; vT is PE-transposed per 128-tile
   (both heads in one transpose) into v [T, 64]-per-head bf16 tiles with an
   appended ones column.
 - attention in S^T orientation: S^T[tk, tq] = kT.T@qT tiles [128, 1024]
   (both heads side by side) in PSUM; softmax without max-subtraction (|S|
   small, safe in fp32): P^T = exp(S^T) on ScalarE (PSUM->SBUF, bf16).
   Causal mask applied only on the 128-wide diagonal blocks by multiplying
   P^T with a precomputed 0/1 triangular bf16 tile on the vector engine (no
   PE mask matmuls). O-matmul lhsT = [v_h | ones] (M=65) yields both
   O^T[d, tq] and the denominator row l in one pass. Normalize via DVE
   reciprocal + gpsimd partition_broadcast + DVE multiply (no PE broadcast
   matmul).
 - the attention loop is software-pipelined (S of tile t+1 ahead of O of
   tile t, double-buffered PSUM) and the next batch's x loads / qkv chains
   are interleaved per tq-chunk so the tensor engine stays busy during the
   scalar-engine-bound stretches.
 - per-batch AllToAll (1 MB/rank) reshards O^T from head-shards to
   token-shards; column-parallel out-projection with fused bias produces
   out^T [C, 1024 tokens] per core; host reassembles. Projections are
   deferred one batch so the tensor engine never waits on a collective.
"""
import numpy as np
import ml_dtypes
import concourse.bacc as bacc
import concourse.mybir as mybir
import concourse.tile as tile
from concourse.bass_utils import run_bass_kernel_spmd
from concourse.masks import make_identity

F32 = mybir.dt.float32
F32R = mybir.dt.float32r
BF16 = mybir.dt.bfloat16
Exp = mybir.ActivationFunctionType.Exp

NCORES = 8
B, T, C, H = 4, 2048, 1024, 16
HD = C // H          # 64
HL = H // NCORES     # 2 heads per core
D2 = HL * HD         # 128 rows of local head-pair dims
TB = T               # tokens per batch
NKC = C // 128       # 8 contraction chunks
NCH = TB // 512      # 4 tq chunks per batch
NTK = TB // 128      # 16 tk tiles per batch
PIECE = TB // NCORES  # 256 tokens per (batch, core) piece after AllToAll

_CACHE = {}


def _build(sim=False):
    nc = bacc.Bacc("TRN2", target_bir_lowering=False, debug=False,
                   num_devices=1 if sim else NCORES)
    xt = nc.dram_tensor("xt", [C, B * T], BF16, kind="ExternalInput").ap()
    wq = nc.dram_tensor("wq", [C, D2], BF16, kind="ExternalInput").ap()
    wk = nc.dram_tensor("wk", [C, D2], BF16, kind="ExternalInput").ap()
    wv = nc.dram_tensor("wv", [C, D2], BF16, kind="ExternalInput").ap()
    wp = nc.dram_tensor("wp", [C, C], F32R, kind="ExternalInput").ap()
    bqkv = nc.dram_tensor("bqkv", [D2, 3], F32, kind="ExternalInput").ap()
    bp = nc.dram_tensor("bp", [128, NKC], F32, kind="ExternalInput").ap()
    outp = nc.dram_tensor("outp", [C, B * PIECE], F32, kind="ExternalOutput").ap()

    inb = [nc.dram_tensor(f"inb{b}", [NCORES, D2, PIECE], F32R) for b in range(B)]
    outb = [nc.dram_tensor(f"outb{b}", [NCORES, D2, PIECE], F32R) for b in range(B)]

    with tile.TileContext(nc) as tc:
        with (
            tc.tile_pool(name="const", bufs=1) as cpool,
            tc.tile_pool(name="w", bufs=1) as wpool,
            tc.tile_pool(name="xt", bufs=5) as xpool,
            tc.tile_pool(name="qk", bufs=2) as qkpool,
            tc.tile_pool(name="vstg", bufs=2) as vstgpool,
            tc.tile_pool(name="vh", bufs=2) as vhpool,
            tc.tile_pool(name="pt", bufs=5) as ptpool,
            tc.tile_pool(name="small", bufs=2) as smallpool,
            tc.tile_pool(name="ofin", bufs=4) as ofinpool,
            tc.tile_pool(name="ot", bufs=1) as otpool,
            tc.tile_pool(name="proj", bufs=1) as projpool,
            tc.tile_pool(name="mm", bufs=2, space="PSUM") as mmps,
            tc.tile_pool(name="s", bufs=2, space="PSUM") as sps,
            tc.tile_pool(name="o", bufs=1, space="PSUM") as ops,
        ):
            # ---- constants ----
            ident32 = cpool.tile([128, 128], F32)
            make_identity(nc, ident32[:])
            identb = cpool.tile([128, 128], BF16)
            tri32 = cpool.tile([128, 128], F32)
            trib = cpool.tile([128, 128], BF16)
            onesb = cpool.tile([128, 16], BF16)
            nc.gpsimd.memset(onesb[:], 1.0)
            nc.gpsimd.memset(tri32[:], 1.0)
            # keep where tq_local >= tk_local (lower-left in S^T layout)
            nc.gpsimd.affine_select(
                out=tri32[:], in_=tri32[:],
                compare_op=mybir.AluOpType.is_ge, fill=0.0,
                base=0, channel_multiplier=-1,
                pattern=[[1, 128]],
            )
            with nc.allow_low_precision(reason="bf16 constant staging"):
                nc.vector.tensor_copy(identb[:], ident32[:])
                nc.vector.tensor_copy(trib[:], tri32[:])

            # ---- weights ----
            # wv first (the first qkv chain is v); biases off the SP queue so
            # they don't delay the startup-critical x load issue
            wq_sb = wpool.tile([128, NKC, D2], BF16)
            wk_sb = wpool.tile([128, NKC, D2], BF16)
            wv_sb = wpool.tile([128, NKC, D2], BF16)
            nc.sync.dma_start(
                wv_sb[:], wv.rearrange("(kc p) m -> p kc m", p=128))
            bqkv_sb = cpool.tile([D2, 3], F32)
            nc.gpsimd.dma_start(bqkv_sb[:], bqkv)
            bp_sb = cpool.tile([128, NKC], F32)
            nc.gpsimd.dma_start(bp_sb[:], bp)

            def emit_x_load(b, n, split=False):
                xts = xpool.tile([128, NKC, 512], BF16, tag="x")
                src = xt[:, b * TB + 512 * n:b * TB + 512 * (n + 1)].rearrange(
                    "(kc p) m -> p kc m", p=128)
                if split:
                    # spread the latency-critical first load across queues,
                    # issued from the idle scalar engine
                    for kc in range(0, NKC, 2):
                        nc.scalar.dma_start(xts[:, kc:kc + 2, :],
                                            src[:, kc:kc + 2, :])
                else:
                    nc.sync.dma_start(xts[:], src)
                return xts

            def alloc_qkv(b):
                qT = qkpool.tile([D2, TB], BF16, tag="qT")
                kT = qkpool.tile([D2, TB], BF16, tag="kT")
                vT = vstgpool.tile([D2, TB], BF16, tag="vT")
                vh = [vhpool.tile([128, NTK * 65], BF16, tag=f"vh{h}",
                                  name=f"vh{h}") for h in range(HL)]
                for h in range(HL):
                    nc.gpsimd.tensor_copy(vh[h][:, 64::65], onesb[:])
                return qT, kT, vT, vh

            def emit_qkv_chain(st, n, col, part=None, cell=None, alt=False):
                xts, qT, kT, vT, vh = st[:5]
                w_sb = (wq_sb, wk_sb, wv_sb)[col]
                if part in (None, 0):
                    cell = cell if cell is not None else {}
                    if alt:
                        # prologue only: borrow the (still idle) attention
                        # score pool so chains don't serialize on evictions
                        big = sps.tile([128, 1024], F32, tag="s",
                                       name="qkv_ps_s")
                        cell['ps'] = big[:, 0:512]
                    else:
                        ps_t = mmps.tile([128, 512], F32, tag="ps",
                                         name="qkv_ps")
                        cell['ps'] = ps_t[:]
                ps = cell['ps']
                kcs = (range(NKC) if part is None else
                       range(part * NKC // 2, (part + 1) * NKC // 2))
                for kc in kcs:
                    nc.tensor.matmul(
                        ps, w_sb[:, kc, :],
                        xts[n][:, kc, :], start=(kc == 0),
                        stop=(kc == NKC - 1))
                if part in (None, 1):
                    dst = (qT, kT, vT)[col]
                    with nc.allow_low_precision(reason="bf16 qkv"):
                        nc.vector.tensor_scalar_add(
                            dst[:, 512 * n:512 * (n + 1)], ps,
                            bqkv_sb[:, col:col + 1])

            def emit_v_transpose(st, tk):
                # both heads in one PE transpose per tk tile
                vT, vh = st[3], st[4]
                vt_ps = mmps.tile([128, 128], BF16, tag="ps", name="vt_ps")
                nc.tensor.transpose(
                    vt_ps[:], vT[:, 128 * tk:128 * (tk + 1)], identb[:])
                for h in range(HL):
                    # (DVE, not gpsimd: GPSIMD cannot access PSUM)
                    with nc.allow_low_precision(reason="bf16 v"):
                        nc.vector.tensor_copy(
                            vh[h][:, 65 * tk:65 * tk + 64],
                            vt_ps[:, 64 * h:64 * (h + 1)])

            def qkv_units(st):
                """Filler units: half a qkv chain or one transpose each,
                chunk-major. v first so its eviction (DVE) overlaps the q/k
                chains."""
                for n in range(NCH):
                    for col in (2, 0, 1):
                        cell = {}
                        for part in (0, 1):
                            yield (lambda n=n, col=col, part=part, cell=cell:
                                   emit_qkv_chain(st, n, col, part, cell))
                    for tk in range(4 * n, 4 * n + 4):
                        yield lambda tk=tk: emit_v_transpose(st, tk)

            def emit_qkv_chunk(st, n):
                for col in (2, 0, 1):
                    emit_qkv_chain(st, n, col)
                for tk in range(4 * n, 4 * n + 4):
                    emit_v_transpose(st, tk)

            def emit_attention(b, qT, kT, vh, units):
                units = list(units)
                n_tiles = sum(4 * j + 4 for j in range(NCH))
                tile_i = 0
                emitted = 0

                def pace():
                    nonlocal emitted
                    want = (tile_i * len(units)) // n_tiles
                    while emitted < want:
                        units[emitted]()
                        emitted += 1

                for j in range(NCH):
                    o_ps = [ops.tile([65, 512], F32, tag=f"o{h}", name=f"o{h}")
                            for h in range(HL)]
                    ktop = 4 * j + 4
                    pts = [None] * ktop  # (pt, z) per tile

                    def emit_o(t):
                        pt, z = pts[t]
                        for h in range(HL):
                            nc.tensor.matmul(
                                o_ps[h][0:65, z:512],
                                vh[h][:, 65 * t:65 * (t + 1)],
                                pt[:, 512 * h + z:512 * (h + 1)],
                                start=(t == 0), stop=(t == ktop - 1))

                    for t in range(ktop):
                        m = t - 4 * j
                        z = 128 * m if m > 0 else 0
                        s_ps = sps.tile([128, 1024], F32, tag="s")
                        for h in range(HL):
                            nc.tensor.matmul(
                                s_ps[:, 512 * h + z:512 * (h + 1)],
                                kT[64 * h:64 * (h + 1),
                                   128 * t:128 * (t + 1)],
                                qT[64 * h:64 * (h + 1),
                                   512 * j + z:512 * (j + 1)],
                                start=True, stop=True)
                        pt = ptpool.tile([128, 1024], BF16, tag="pt")
                        if z:
                            exp_src = s_ps[:].rearrange(
                                "p (g c) -> p g c", g=2)[:, :, z:]
                            exp_dst = pt[:].rearrange(
                                "p (g c) -> p g c", g=2)[:, :, z:]
                            nc.scalar.activation(exp_dst, exp_src, Exp)
                        else:
                            nc.scalar.activation(pt[:], s_ps[:], Exp)
                        if m >= 0:
                            # zero the upper triangle of the diagonal block
                            for h in range(HL):
                                nc.vector.tensor_mul(
                                    pt[:, 512 * h + z:512 * h + z + 128],
                                    pt[:, 512 * h + z:512 * h + z + 128],
                                    trib[:])
                        pts[t] = (pt, z)
                        if t >= 1:
                            emit_o(t - 1)
                        tile_i += 1
                        pace()
                    emit_o(ktop - 1)

                    # ---- normalization ----
                    last = b == B - 1 and j == NCH - 1
                    for h in range(HL):
                        if last:
                            # tail-latency path: no successor needs the PSUM
                            # bank, so skip the eviction copy
                            o_sb = o_ps[h]
                        else:
                            # evict PSUM immediately so the next chunk's O
                            # accumulation isn't blocked on the norm chain
                            o_sb = smallpool.tile([65, 512], F32, tag="osb")
                            nc.vector.tensor_copy(o_sb[:], o_ps[h][:])
                        r_sb = smallpool.tile([1, 512], F32, tag="r")
                        nc.vector.reciprocal(r_sb[:], o_sb[64:65, :])
                        rb_sb = smallpool.tile([64, 512], F32, tag="rb")
                        nc.gpsimd.partition_broadcast(rb_sb[:], r_sb[:])
                        ofin = ofinpool.tile([64, 512], F32R)
                        with nc.allow_low_precision(reason="f32r O"):
                            nc.vector.tensor_mul(ofin[:], o_sb[0:64, :],
                                                 rb_sb[:])
                        for half in range(2):
                            s8 = 2 * j + half
                            nc.sync.dma_start(
                                inb[b].ap()[s8, 64 * h:64 * (h + 1), :],
                                ofin[:, 256 * half:256 * (half + 1)])
                while emitted < len(units):
                    units[emitted]()
                    emitted += 1

            def emit_collective(b):
                if sim:
                    # stand-in for the cost-model sim: the resharding data
                    # movement as per-destination slice copies (parallel DMA
                    # rings, like the real AllToAll); wire time excluded
                    for s8 in range(NCORES):
                        nc.gpsimd.dma_start(outb[b].ap()[s8], inb[b].ap()[s8])
                else:
                    nc.gpsimd.collective_compute(
                        "AllToAll", mybir.AluOpType.bypass,
                        replica_groups=[list(range(NCORES))],
                        ins=[inb[b].ap().opt()], outs=[outb[b].ap().opt()],
                    )

            def emit_proj_load(bs, split=False):
                nb = len(bs)
                tag = "ot" + "".join(map(str, bs))
                ot = otpool.tile([128, NCORES, nb * PIECE], F32R, tag=tag)
                for i in range(nb):
                    src = outb[bs[i]].ap().rearrange("s p m -> p s m")
                    if split:
                        # tail-latency path: parallel queues, issued from the
                        # idle scalar engine
                        for s8 in range(0, NCORES, 2):
                            nc.scalar.dma_start(
                                ot[:, s8:s8 + 2, PIECE * i:PIECE * (i + 1)],
                                src[:, s8:s8 + 2, :])
                    else:
                        nc.sync.dma_start(
                            ot[:, :, PIECE * i:PIECE * (i + 1)], src)
                osb = projpool.tile([128, NKC, nb * PIECE], F32,
                                    tag="osb" + "".join(map(str, bs)))
                return ot, osb

            def emit_proj_cols(bs, ot, osb, mcols, tail=False):
                # out-projection matmuls for one or two batches (512-wide
                # when two)
                w = len(bs) * PIECE
                for mcol in mcols:
                    pp = mmps.tile([128, 512], F32, tag="ps", name="pp")
                    for s8 in range(NCORES):
                        nc.tensor.matmul(
                            pp[:, 0:w],
                            wp_sb[:, s8, 128 * mcol:128 * (mcol + 1)],
                            ot[:, s8, :], start=(s8 == 0),
                            stop=(s8 == NCORES - 1))
                    # (DVE, not gpsimd: GPSIMD cannot access PSUM)
                    nc.vector.tensor_scalar_add(osb[:, mcol, :], pp[:, 0:w],
                                                bp_sb[:, mcol:mcol + 1])

            def emit_proj_store(bs, osb, nsplit=1, tail=False):
                # split the store across queues to shorten the tail
                eng = nc.scalar if tail else nc.sync
                for i in range(len(bs)):
                    dst = outp[:, PIECE * bs[i]:PIECE * (bs[i] + 1)].rearrange(
                        "(mc p) m -> p mc m", p=128)
                    step = NKC // nsplit
                    for k in range(nsplit):
                        eng.dma_start(
                            dst[:, k * step:(k + 1) * step, :],
                            osb[:, k * step:(k + 1) * step,
                                PIECE * i:PIECE * (i + 1)])

            def emit_proj(bs):
                ot, osb = emit_proj_load(bs)
                emit_proj_cols(bs, ot, osb, range(NKC))
                emit_proj_store(bs, osb)

            # ---- main schedule ----
            states = [None] * B

            def make_state(b):
                xts = [emit_x_load(b, n, split=(b == 0 and n == 0))
                       for n in range(NCH)]
                return [xts, *alloc_qkv(b)]

            xts0 = [emit_x_load(0, 0, split=True)]
            for t, d in ((wq_sb, wq), (wk_sb, wk)):
                nc.sync.dma_start(
                    t[:], d.rearrange("(kc p) m -> p kc m", p=128))
            xts0.append(emit_x_load(0, 1, split=True))
            xts0 += [emit_x_load(0, n) for n in range(2, NCH)]
            states[0] = [xts0, *alloc_qkv(0)]
            for n in range(NCH):
                for i, col in enumerate((2, 0, 1)):
                    emit_qkv_chain(states[0], n, col, alt=(i % 2 == 1))
                for tk in range(4 * n, 4 * n + 4):
                    emit_v_transpose(states[0], tk)
            states[1] = make_state(1)
            # W_proj load deferred past the latency-critical start
            wp_sb = wpool.tile([128, NKC, C], F32R)
            nc.sync.dma_start(
                wp_sb[:], wp.rearrange("(kc p) m -> p kc m", p=128))

            proj01 = [None, None]
            proj2 = [None, None]
            for b in range(B):
                if b + 1 < B:
                    units = qkv_units(states[b + 1])
                else:
                    # spread the deferred projections of batches 0-2 into
                    # batch 3's attention as tensor-engine gap filler
                    units = (
                        (lambda mcol=mcol: emit_proj_cols(
                            [0, 1], proj01[0], proj01[1], [mcol]))
                        for mcol in range(NKC))
                st = states[b]
                emit_attention(b, st[1], st[2], st[4], units)
                emit_collective(b)
                if b + 2 < B:
                    states[b + 2] = make_state(b + 2)
                if b == 1:
                    proj01[0], proj01[1] = emit_proj_load([0, 1])
                elif b == 2:
                    # preload the batch-2 half of the tail projection input
                    ot23 = otpool.tile([128, NCORES, 2 * PIECE], F32R,
                                       tag="ot23")
                    nc.sync.dma_start(
                        ot23[:, :, 0:PIECE],
                        outb[2].ap().rearrange("s p m -> p s m"))
            emit_proj_store([0, 1], proj01[1])
            src3 = outb[3].ap().rearrange("s p m -> p s m")
            for s8 in range(0, NCORES, 2):
                nc.scalar.dma_start(
                    ot23[:, s8:s8 + 2, PIECE:2 * PIECE], src3[:, s8:s8 + 2, :])
            osb23 = projpool.tile([128, NKC, 2 * PIECE], F32, tag="osb23")
            emit_proj_cols([2, 3], ot23, osb23, range(NKC), tail=True)
            emit_proj_store([2, 3], osb23, nsplit=4, tail=True)
    nc.compile()
    return nc


def _get_nc():
    if "nc" not in _CACHE:
        _CACHE["nc"] = _build()
    return _CACHE["nc"]


def kernel(x, W_qkv, b_qkv, W_proj, b_proj):
    x = np.asarray(x, dtype=np.float32)
    W_qkv = np.asarray(W_qkv, dtype=np.float32)
    b_qkv = np.asarray(b_qkv, dtype=np.float32)
    W_proj = np.asarray(W_proj, dtype=np.float32)
    b_proj = np.asarray(b_proj, dtype=np.float32)

    scale = 1.0 / np.sqrt(HD)
    BFD = ml_dtypes.bfloat16
    xtb = np.ascontiguousarray(x.reshape(B * T, C).T).astype(BFD)  # [C, B*T]
    wp = np.ascontiguousarray(W_proj)                          # [C, C]
    bpz = np.ascontiguousarray(b_proj.reshape(NKC, 128).T)     # [128, 8]

    qw = W_qkv[:, 0:C]
    kw = W_qkv[:, C:2 * C]
    vw = W_qkv[:, 2 * C:3 * C]
    qb, kb, vb = b_qkv[0:C], b_qkv[C:2 * C], b_qkv[2 * C:3 * C]

    in_maps = []
    for c in range(NCORES):
        cols = slice(2 * c * HD, (2 * c + 2) * HD)  # this core's 128 dims
        bq = np.stack([qb[cols] * scale, kb[cols], vb[cols]], axis=1)  # [128,3]
        in_maps.append({
            "xt": xtb,
            "wq": np.ascontiguousarray(qw[:, cols] * scale).astype(BFD),
            "wk": np.ascontiguousarray(kw[:, cols]).astype(BFD),
            "wv": np.ascontiguousarray(vw[:, cols]).astype(BFD),
            "wp": wp,
            "bqkv": np.ascontiguousarray(bq),
            "bp": bpz,
        })

    nc = _get_nc()
    _CACHE["last_in_maps"] = in_maps
    res = run_bass_kernel_spmd(nc, in_maps, core_ids=list(range(NCORES)))

    # outp[c]: [C, B*PIECE] (cols: b-major, then 256 tokens of piece c)
    allo = np.stack([res.results[c]["outp"] for c in range(NCORES)])
    allo = allo.reshape(NCORES, C, B, PIECE)       # [c, ch, b, u]
    out = allo.transpose(2, 0, 3, 1).reshape(B, T, C)
    return np.ascontiguousarray(out)


# revision 49
# speedup vs baseline: 1.5642x; 1.0028x over previous
"""Causal self-attention (B=4, T=2048, C=1024, H=16) on 8 trn2 NeuronCores.

Sharding: head-pair parallel. Core c owns heads {2c, 2c+1} for all 4 batches.
 - host: x is pre-transposed to xT [C, B*T] (bf16); W_qkv is pre-sliced per
   core into wq/wk/wv [C, 128] bf16 (softmax scale folded into wq), W_proj
   (f32r) and biases broadcast.
 - device per core: qkv projections as bf16 matmuls producing qT/kT [d2, T]
   (d on partitions, bf16) and vT [d2, T]; vT is PE-transposed per 128-tile
   (both heads in one transpose) into v [T, 64]-per-head bf16 tiles with an
   appended ones column.
 - attention in S^T orientation: S^T[tk, tq] = kT.T@qT tiles [128, 1024]
   (both heads side by side) in PSUM; softmax without max-subtraction (|S|
   small, safe in fp32): P^T = exp(S^T) on ScalarE (PSUM->SBUF, bf16).
   Causal mask applied only on the 128-wide diagonal blocks by multiplying
   P^T with a precomputed 0/1 triangular bf16 tile on the vector engine (no
   PE mask matmuls). O-matmul lhsT = [v_h | ones] (M=65) yields both
   O^T[d, tq] and the denominator row l in one pass. Normalize via DVE
   reciprocal + gpsimd partition_broadcast + DVE multiply (no PE broadcast
   matmul).
 - the attention loop is software-pipelined (S of tile t+1 ahead of O of
   tile t, double-buffered PSUM) and the next batch's x loads / qkv chains
   are paced into the tile loop as filler units so the tensor engine stays
   busy during the scalar-engine-bound stretches.
 - per-batch AllToAll (1 MB/rank) reshards O^T from head-shards to
   token-shards; column-parallel out-projection with fused bias produces
   out^T [C, 1024 tokens] per core; host reassembles. Projections are
   deferred and run as 512-wide batch-pairs: (0,1) fills batch 3's
   attention, (2,3) forms the tail with latency-split DMAs, so the tensor
   engine never waits on a collective mid-run.
"""
import numpy as np
import ml_dtypes
import concourse.bacc as bacc
import concourse.mybir as mybir
import concourse.tile as tile
from concourse.bass_utils import run_bass_kernel_spmd
from concourse.masks import make_identity

F32 = mybir.dt.float32
F32R = mybir.dt.float32r
BF16 = mybir.dt.bfloat16
Exp = mybir.ActivationFunctionType.Exp

NCORES = 8
B, T, C, H = 4, 2048, 1024, 16
HD = C // H          # 64
HL = H // NCORES     # 2 heads per core
D2 = HL * HD         # 128 rows of local head-pair dims
TB = T               # tokens per batch
NKC = C // 128       # 8 contraction chunks
NCH = TB // 512      # 4 tq chunks per batch
NTK = TB // 128      # 16 tk tiles per batch
PIECE = TB // NCORES  # 256 tokens per (batch, core) piece after AllToAll

_CACHE = {}


def _build(sim=False):
    nc = bacc.Bacc("TRN2", target_bir_lowering=False, debug=False,
                   num_devices=1 if sim else NCORES)
    xt = nc.dram_tensor("xt", [C, B * T], BF16, kind="ExternalInput").ap()
    wq = nc.dram_tensor("wq", [C, D2], BF16, kind="ExternalInput").ap()
    wk = nc.dram_tensor("wk", [C, D2], BF16, kind="ExternalInput").ap()
    wv = nc.dram_tensor("wv", [C, D2], BF16, kind="ExternalInput").ap()
    wp = nc.dram_tensor("wp", [C, C], F32R, kind="ExternalInput").ap()
    bqkv = nc.dram_tensor("bqkv", [D2, 3], F32, kind="ExternalInput").ap()
    bp = nc.dram_tensor("bp", [128, NKC], F32, kind="ExternalInput").ap()
    outp = nc.dram_tensor("outp", [C, B * PIECE], F32, kind="ExternalOutput").ap()

    inb = [nc.dram_tensor(f"inb{b}", [NCORES, D2, PIECE], F32R) for b in range(B)]
    outb = [nc.dram_tensor(f"outb{b}", [NCORES, D2, PIECE], F32R) for b in range(B)]

    with tile.TileContext(nc) as tc:
        with (
            tc.tile_pool(name="const", bufs=1) as cpool,
            tc.tile_pool(name="w", bufs=1) as wpool,
            tc.tile_pool(name="xt", bufs=4) as xpool,
            tc.tile_pool(name="qk", bufs=2) as qkpool,
            tc.tile_pool(name="vstg", bufs=2) as vstgpool,
            tc.tile_pool(name="vh", bufs=2) as vhpool,
            tc.tile_pool(name="pt", bufs=6) as ptpool,
            tc.tile_pool(name="small", bufs=3) as smallpool,
            tc.tile_pool(name="ofin", bufs=4) as ofinpool,
            tc.tile_pool(name="ot", bufs=1) as otpool,
            tc.tile_pool(name="proj", bufs=1) as projpool,
            tc.tile_pool(name="mm", bufs=2, space="PSUM") as mmps,
            tc.tile_pool(name="s", bufs=2, space="PSUM") as sps,
            tc.tile_pool(name="o", bufs=1, space="PSUM") as ops,
        ):
            # ---- constants ----
            ident32 = cpool.tile([128, 128], F32)
            make_identity(nc, ident32[:])
            identb = cpool.tile([128, 128], BF16)
            tri32 = cpool.tile([128, 128], F32)
            trib = cpool.tile([128, 128], BF16)
            onesb = cpool.tile([128, 16], BF16)
            nc.gpsimd.memset(onesb[:], 1.0)
            nc.gpsimd.memset(tri32[:], 1.0)
            # keep where tq_local >= tk_local (lower-left in S^T layout)
            nc.gpsimd.affine_select(
                out=tri32[:], in_=tri32[:],
                compare_op=mybir.AluOpType.is_ge, fill=0.0,
                base=0, channel_multiplier=-1,
                pattern=[[1, 128]],
            )
            with nc.allow_low_precision(reason="bf16 constant staging"):
                nc.vector.tensor_copy(identb[:], ident32[:])
                nc.vector.tensor_copy(trib[:], tri32[:])

            # ---- weights ----
            # wv first (the first qkv chain is v); biases off the SP queue so
            # they don't delay the startup-critical x load issue
            wq_sb = wpool.tile([128, NKC, D2], BF16)
            wk_sb = wpool.tile([128, NKC, D2], BF16)
            wv_sb = wpool.tile([128, NKC, D2], BF16)
            nc.sync.dma_start(
                wv_sb[:], wv.rearrange("(kc p) m -> p kc m", p=128))
            bqkv_sb = cpool.tile([D2, 3], F32)
            nc.gpsimd.dma_start(bqkv_sb[:], bqkv)
            bp_sb = cpool.tile([128, NKC], F32)
            nc.gpsimd.dma_start(bp_sb[:], bp)

            def emit_x_load(b, n, split=False):
                xts = xpool.tile([128, NKC, 512], BF16, tag="x")
                src = xt[:, b * TB + 512 * n:b * TB + 512 * (n + 1)].rearrange(
                    "(kc p) m -> p kc m", p=128)
                if split:
                    # spread the latency-critical first load across queues,
                    # issued from the idle scalar engine
                    for kc in range(0, NKC, 2):
                        nc.scalar.dma_start(xts[:, kc:kc + 2, :],
                                            src[:, kc:kc + 2, :])
                else:
                    nc.sync.dma_start(xts[:], src)
                return xts

            def alloc_qkv(b):
                qT = qkpool.tile([D2, TB], BF16, tag="qT")
                kT = qkpool.tile([D2, TB], BF16, tag="kT")
                vT = vstgpool.tile([D2, TB], BF16, tag="vT")
                vh = [vhpool.tile([128, NTK * 65], BF16, tag=f"vh{h}",
                                  name=f"vh{h}") for h in range(HL)]
                for h in range(HL):
                    nc.gpsimd.tensor_copy(vh[h][:, 64::65], onesb[:])
                return qT, kT, vT, vh

            def emit_qkv_chain(st, n, col, part=None, cell=None, alt=False):
                xts, qT, kT, vT, vh = st[:5]
                w_sb = (wq_sb, wk_sb, wv_sb)[col]
                if part in (None, 0):
                    cell = cell if cell is not None else {}
                    if alt:
                        # prologue only: borrow the (still idle) attention
                        # score pool so chains don't serialize on evictions
                        big = sps.tile([128, 1024], F32, tag="s",
                                       name="qkv_ps_s")
                        cell['ps'] = big[:, 0:512]
                    else:
                        ps_t = mmps.tile([128, 512], F32, tag="ps",
                                         name="qkv_ps")
                        cell['ps'] = ps_t[:]
                ps = cell['ps']
                kcs = (range(NKC) if part is None else
                       range(part * NKC // 2, (part + 1) * NKC // 2))
                for kc in kcs:
                    nc.tensor.matmul(
                        ps, w_sb[:, kc, :],
                        xts[n][:, kc, :], start=(kc == 0),
                        stop=(kc == NKC - 1))
                if part in (None, 1):
                    dst = (qT, kT, vT)[col]
                    with nc.allow_low_precision(reason="bf16 qkv"):
                        nc.vector.tensor_scalar_add(
                            dst[:, 512 * n:512 * (n + 1)], ps,
                            bqkv_sb[:, col:col + 1])

            def emit_v_transpose(st, tk):
                # both heads in one PE transpose per tk tile
                vT, vh = st[3], st[4]
                vt_ps = mmps.tile([128, 128], BF16, tag="ps", name="vt_ps")
                nc.tensor.transpose(
                    vt_ps[:], vT[:, 128 * tk:128 * (tk + 1)], identb[:])
                for h in range(HL):
                    # (DVE, not gpsimd: GPSIMD cannot access PSUM)
                    with nc.allow_low_precision(reason="bf16 v"):
                        nc.vector.tensor_copy(
                            vh[h][:, 65 * tk:65 * tk + 64],
                            vt_ps[:, 64 * h:64 * (h + 1)])

            def qkv_units(st):
                """Filler units: half a qkv chain or one transpose each,
                chunk-major. v first so its eviction (DVE) overlaps the q/k
                chains."""
                for n in range(NCH):
                    for col in (2, 0, 1):
                        cell = {}
                        for part in (0, 1):
                            yield (lambda n=n, col=col, part=part, cell=cell:
                                   emit_qkv_chain(st, n, col, part, cell))
                    for tk in range(4 * n, 4 * n + 4):
                        yield lambda tk=tk: emit_v_transpose(st, tk)

            def emit_attention(b, qT, kT, vh, units):
                units = list(units)
                n_tiles = sum(4 * j + 4 for j in range(NCH))
                tile_i = 0
                emitted = 0

                def pace():
                    nonlocal emitted
                    want = (tile_i * len(units)) // n_tiles
                    while emitted < want:
                        units[emitted]()
                        emitted += 1

                for j in range(NCH):
                    o_ps = [ops.tile([65, 512], F32, tag=f"o{h}", name=f"o{h}")
                            for h in range(HL)]
                    ktop = 4 * j + 4
                    pts = [None] * ktop  # (pt, z) per tile

                    def emit_o(t):
                        pt, z = pts[t]
                        for h in range(HL):
                            nc.tensor.matmul(
                                o_ps[h][0:65, z:512],
                                vh[h][:, 65 * t:65 * (t + 1)],
                                pt[:, 512 * h + z:512 * (h + 1)],
                                start=(t == 0), stop=(t == ktop - 1))

                    for t in range(ktop):
                        m = t - 4 * j
                        z = 128 * m if m > 0 else 0
                        s_ps = sps.tile([128, 1024], F32, tag="s")
                        for h in range(HL):
                            nc.tensor.matmul(
                                s_ps[:, 512 * h + z:512 * (h + 1)],
                                kT[64 * h:64 * (h + 1),
                                   128 * t:128 * (t + 1)],
                                qT[64 * h:64 * (h + 1),
                                   512 * j + z:512 * (j + 1)],
                                start=True, stop=True)
                        pt = ptpool.tile([128, 1024], BF16, tag="pt")
                        if z:
                            exp_src = s_ps[:].rearrange(
                                "p (g c) -> p g c", g=2)[:, :, z:]
                            exp_dst = pt[:].rearrange(
                                "p (g c) -> p g c", g=2)[:, :, z:]
                            nc.scalar.activation(exp_dst, exp_src, Exp)
                        else:
                            nc.scalar.activation(pt[:], s_ps[:], Exp)
                        if m >= 0:
                            # zero the upper triangle of the diagonal block
                            for h in range(HL):
                                nc.vector.tensor_mul(
                                    pt[:, 512 * h + z:512 * h + z + 128],
                                    pt[:, 512 * h + z:512 * h + z + 128],
                                    trib[:])
                        pts[t] = (pt, z)
                        if t >= 1:
                            emit_o(t - 1)
                        tile_i += 1
                        pace()
                    emit_o(ktop - 1)

                    # ---- normalization ----
                    last = b == B - 1 and j == NCH - 1
                    for h in range(HL):
                        if last:
                            # tail-latency path: no successor needs the PSUM
                            # bank, so skip the eviction copy
                            o_sb = o_ps[h]
                        else:
                            # evict PSUM immediately so the next chunk's O
                            # accumulation isn't blocked on the norm chain
                            o_sb = smallpool.tile([65, 512], F32, tag="osb")
                            nc.vector.tensor_copy(o_sb[:], o_ps[h][:])
                        r_sb = smallpool.tile([1, 512], F32, tag="r")
                        nc.vector.reciprocal(r_sb[:], o_sb[64:65, :])
                        rb_sb = smallpool.tile([64, 512], F32, tag="rb")
                        nc.gpsimd.partition_broadcast(rb_sb[:], r_sb[:])
                        ofin = ofinpool.tile([64, 512], F32R)
                        with nc.allow_low_precision(reason="f32r O"):
                            nc.vector.tensor_mul(ofin[:], o_sb[0:64, :],
                                                 rb_sb[:])
                        for half in range(2):
                            s8 = 2 * j + half
                            nc.sync.dma_start(
                                inb[b].ap()[s8, 64 * h:64 * (h + 1), :],
                                ofin[:, 256 * half:256 * (half + 1)])
                while emitted < len(units):
                    units[emitted]()
                    emitted += 1

            def emit_collective(b):
                if sim:
                    # stand-in for the cost-model sim: the resharding data
                    # movement as per-destination slice copies (parallel DMA
                    # rings, like the real AllToAll); wire time excluded
                    for s8 in range(NCORES):
                        nc.gpsimd.dma_start(outb[b].ap()[s8], inb[b].ap()[s8])
                else:
                    nc.gpsimd.collective_compute(
                        "AllToAll", mybir.AluOpType.bypass,
                        replica_groups=[list(range(NCORES))],
                        ins=[inb[b].ap().opt()], outs=[outb[b].ap().opt()],
                    )

            def emit_proj_load(bs, split=False):
                nb = len(bs)
                tag = "ot" + "".join(map(str, bs))
                ot = otpool.tile([128, NCORES, nb * PIECE], F32R, tag=tag)
                for i in range(nb):
                    src = outb[bs[i]].ap().rearrange("s p m -> p s m")
                    if split:
                        # tail-latency path: parallel queues, issued from the
                        # idle scalar engine
                        for s8 in range(0, NCORES, 2):
                            nc.scalar.dma_start(
                                ot[:, s8:s8 + 2, PIECE * i:PIECE * (i + 1)],
                                src[:, s8:s8 + 2, :])
                    else:
                        nc.sync.dma_start(
                            ot[:, :, PIECE * i:PIECE * (i + 1)], src)
                osb = projpool.tile([128, NKC, nb * PIECE], F32,
                                    tag="osb" + "".join(map(str, bs)))
                return ot, osb

            def emit_proj_cols(bs, ot, osb, mcols, tail=False):
                # out-projection matmuls for one or two batches (512-wide
                # when two)
                w = len(bs) * PIECE
                for mcol in mcols:
                    pp = mmps.tile([128, 512], F32, tag="ps", name="pp")
                    for s8 in range(NCORES):
                        nc.tensor.matmul(
                            pp[:, 0:w],
                            wp_sb[:, s8, 128 * mcol:128 * (mcol + 1)],
                            ot[:, s8, :], start=(s8 == 0),
                            stop=(s8 == NCORES - 1))
                    # (DVE, not gpsimd: GPSIMD cannot access PSUM)
                    nc.vector.tensor_scalar_add(osb[:, mcol, :], pp[:, 0:w],
                                                bp_sb[:, mcol:mcol + 1])

            def emit_proj_store(bs, osb, nsplit=1, tail=False):
                # split the store across queues to shorten the tail
                eng = nc.scalar if tail else nc.sync
                for i in range(len(bs)):
                    dst = outp[:, PIECE * bs[i]:PIECE * (bs[i] + 1)].rearrange(
                        "(mc p) m -> p mc m", p=128)
                    step = NKC // nsplit
                    for k in range(nsplit):
                        eng.dma_start(
                            dst[:, k * step:(k + 1) * step, :],
                            osb[:, k * step:(k + 1) * step,
                                PIECE * i:PIECE * (i + 1)])

            # ---- main schedule ----
            states = [None] * B

            def make_state(b):
                xts = [emit_x_load(b, n, split=(b == 0 and n == 0))
                       for n in range(NCH)]
                return [xts, *alloc_qkv(b)]

            xts0 = [emit_x_load(0, 0, split=True)]
            for t, d in ((wq_sb, wq), (wk_sb, wk)):
                nc.sync.dma_start(
                    t[:], d.rearrange("(kc p) m -> p kc m", p=128))
            xts0.append(emit_x_load(0, 1, split=True))
            xts0 += [emit_x_load(0, n) for n in range(2, NCH)]
            states[0] = [xts0, *alloc_qkv(0)]
            for n in range(NCH):
                for i, col in enumerate((2, 0, 1)):
                    emit_qkv_chain(states[0], n, col, alt=(i % 2 == 1))
                for tk in range(4 * n, 4 * n + 4):
                    emit_v_transpose(states[0], tk)
            states[1] = make_state(1)
            # W_proj load deferred past the latency-critical start
            wp_sb = wpool.tile([128, NKC, C], F32R)
            nc.sync.dma_start(
                wp_sb[:], wp.rearrange("(kc p) m -> p kc m", p=128))

            proj01 = [None, None]
            for b in range(B):
                if b + 1 < B:
                    units = qkv_units(states[b + 1])
                else:
                    # spread the deferred projections of batches 0-2 into
                    # batch 3's attention as tensor-engine gap filler
                    units = (
                        (lambda mcol=mcol: emit_proj_cols(
                            [0, 1], proj01[0], proj01[1], [mcol]))
                        for mcol in range(NKC))
                st = states[b]
                emit_attention(b, st[1], st[2], st[4], units)
                emit_collective(b)
                if b + 2 < B:
                    states[b + 2] = make_state(b + 2)
                if b == 1:
                    proj01[0], proj01[1] = emit_proj_load([0, 1])
                elif b == 2:
                    # preload the batch-2 half of the tail projection input
                    ot23 = otpool.tile([128, NCORES, 2 * PIECE], F32R,
                                       tag="ot23")
                    nc.sync.dma_start(
                        ot23[:, :, 0:PIECE],
                        outb[2].ap().rearrange("s p m -> p s m"))
            emit_proj_store([0, 1], proj01[1])
            src3 = outb[3].ap().rearrange("s p m -> p s m")
            for s8 in range(0, NCORES, 2):
                nc.scalar.dma_start(
                    ot23[:, s8:s8 + 2, PIECE:2 * PIECE], src3[:, s8:s8 + 2, :])
            osb23 = projpool.tile([128, NKC, 2 * PIECE], F32, tag="osb23")
            emit_proj_cols([2, 3], ot23, osb23, range(NKC), tail=True)
            emit_proj_store([2, 3], osb23, nsplit=4, tail=True)
    nc.compile()
    return nc


def _get_nc():
    if "nc" not in _CACHE:
        _CACHE["nc"] = _build()
    return _CACHE["nc"]


def kernel(x, W_qkv, b_qkv, W_proj, b_proj):
    x = np.asarray(x, dtype=np.float32)
    W_qkv = np.asarray(W_qkv, dtype=np.float32)
    b_qkv = np.asarray(b_qkv, dtype=np.float32)
    W_proj = np.asarray(W_proj, dtype=np.float32)
    b_proj = np.asarray(b_proj, dtype=np.float32)

    scale = 1.0 / np.sqrt(HD)
    BFD = ml_dtypes.bfloat16
    xtb = np.ascontiguousarray(x.reshape(B * T, C).T).astype(BFD)  # [C, B*T]
    wp = np.ascontiguousarray(W_proj)                          # [C, C]
    bpz = np.ascontiguousarray(b_proj.reshape(NKC, 128).T)     # [128, 8]

    qw = W_qkv[:, 0:C]
    kw = W_qkv[:, C:2 * C]
    vw = W_qkv[:, 2 * C:3 * C]
    qb, kb, vb = b_qkv[0:C], b_qkv[C:2 * C], b_qkv[2 * C:3 * C]

    in_maps = []
    for c in range(NCORES):
        cols = slice(2 * c * HD, (2 * c + 2) * HD)  # this core's 128 dims
        bq = np.stack([qb[cols] * scale, kb[cols], vb[cols]], axis=1)  # [128,3]
        in_maps.append({
            "xt": xtb,
            "wq": np.ascontiguousarray(qw[:, cols] * scale).astype(BFD),
            "wk": np.ascontiguousarray(kw[:, cols]).astype(BFD),
            "wv": np.ascontiguousarray(vw[:, cols]).astype(BFD),
            "wp": wp,
            "bqkv": np.ascontiguousarray(bq),
            "bp": bpz,
        })

    nc = _get_nc()
    _CACHE["last_in_maps"] = in_maps
    res = run_bass_kernel_spmd(nc, in_maps, core_ids=list(range(NCORES)))

    # outp[c]: [C, B*PIECE] (cols: b-major, then 256 tokens of piece c)
    allo = np.stack([res.results[c]["outp"] for c in range(NCORES)])
    allo = allo.reshape(NCORES, C, B, PIECE)       # [c, ch, b, u]
    out = allo.transpose(2, 0, 3, 1).reshape(B, T, C)
    return np.ascontiguousarray(out)

